# revision 8
# baseline (speedup 1.0000x reference)
"""Trainium2 Bass kernel for nn_ErrorAwareEdgeLoss.

reference:  cost[b,e] = sum_{p,q} P[b,i_e,p] * d_error[p,q] * P[b,j_e,q]
            result    = mean_{b,e} cost[b,e]

The edge pairs only enter through the count matrix
    C[l1,l2] = #edges e with (i_e,j_e) == (l1,l2),
and since d_error is symmetric the result collapses to
    result = <d_error, sum_b Q_b^T Cs Q_b> / (B*E),
with Q_b = P[b,:64,:] and Cs = (C + C^T)/2 (exact in fp8: half-integers).

Device work per core (256 batches, data-parallel over batch), all in fp8
(e4m3, host-packed as 64*Q — the softmax values are tiny so 64*Q stays
well inside [2^-6, 240] and the final scale divides by 64^2; empirical
rel err vs f64 reference ~2e-3, tolerance 2e-2):
  - warmup matmuls on a zeroed scratch tile start the PE before any DMA
    lands (the HAM clock gate needs ~3.4us of sustained PE activity to
    lift the PE from 1.2 to 2.4GHz); scratch is memset on GPSIMD, which
    is idle after the preamble, so the PE starts ~0.7us earlier than a
    DVE memset allows
  - cs (blockdiag(Cs,Cs), so one matmul computes both batch-halves' Y)
    is packed as column 0 of the pq dram tensor: the lead-in group load
    brings {cs | first pairs} in ONE descriptor batch on the sync ring,
    saving a ~780ns DIRECT2D on the lead-in critical path
  - group loads all go on the ONE sync HWDGE ring so they complete
    strictly in consumption order; [128, pairs, 128] fp8 tiles, two
    batches per 128 partitions
  - Y = Cs @ Q via ONE blockdiag(Cs,Cs) matmul per 512-wide slab
  - cast Y (PSUM f32) to fp8 in SBUF as whole 512-col slabs alternating
    between DVE and ACT (fixed per-instruction overhead dominates
    smaller casts)
  - R += Q^T Y with K=256 fp8 DoubleRow matmuls (two batch-pairs per
    instruction) accumulated in PSUM f32 — R-matmuls run one group
    behind the Y-matmuls so the PE never waits on the PSUM->SBUF casts
  - final reduce ON DEVICE: s[p] = sum_q R[p,q]*d_error[p,q] via one
    DVE tensor_tensor_reduce (PSUM x SBUF), then a 1-column f32 matmul
    against a ones vector folds partitions, and a single 4-byte result
    is written out — replacing the 64KB R writeback (+copies +big
    descriptor batch) that used to pace the tail.
Host: result = sum_c scalar_c / (B*E*64^2).
"""

import sys

_TRN_REPO = "/opt/trn_rl_repo"
if _TRN_REPO not in sys.path:
    sys.path.insert(0, _TRN_REPO)

import numpy as np
import ml_dtypes

B, L, H = 2048, 64, 128     # batch, logical qubits, physical dim
E = 512                     # number of circuit edges
N_CORES = 8
BPC = B // N_CORES          # 256 batches per core
NPAIRS = BPC // 2           # 128 batch-pairs per core
GROUPS = [8, 8] + [16] * 6 + [8, 8]  # pairs per load: small lead-in groups
                            # start the PE sooner, small tail groups cut
                            # the trailing R run; sum must be NPAIRS
SLAB_PAIRS = 4              # pairs per Y-matmul slab (512 moving columns)
QSCALE = 64.0               # host-side scale before fp8 cast
WARMUP_MM = 22              # dummy matmuls that keep the PE busy (ramping
                            # the HAM clock gate) until the first group
                            # load lands
D_LOAD_AFTER = 7            # ring position of the d_error load (late
                            # enough not to delay the lead-in groups,
                            # early enough to land before the final
                            # reduce)

_CACHE = {}


def _build():
    import concourse.tile as tile
    from concourse import bacc, mybir

    f32 = mybir.dt.float32
    fp8 = mybir.dt.float8e4

    nc = bacc.Bacc(None)
    # host-packed shard: pq[p, 0, h] = blockdiag(Cs,Cs)[p, h] and
    # pq[p, 1+j, h] = 64*Q[2j + p//64, p%64, h] over the core's 128
    # batch-pairs j — each group load is a plain 2D DMA with a
    # contiguous run per partition.
    pq = nc.dram_tensor("pq", [128, 1 + NPAIRS, H], fp8, kind="ExternalInput")
    dmat = nc.dram_tensor("dmat", [H, H], f32, kind="ExternalInput")
    res_out = nc.dram_tensor("res_out", [1, 1], f32, kind="ExternalOutput")

    with tile.TileContext(nc) as tc:
        with (
            tc.tile_pool(name="singles", bufs=1) as singles,
            tc.tile_pool(name="qbfp", bufs=6) as qbf_pool,
            tc.tile_pool(name="ybfp", bufs=3) as ybf_pool,
            tc.tile_pool(name="yps", bufs=6, space="PSUM") as yps,
            tc.tile_pool(name="rps", bufs=1, space="PSUM") as rps,
            tc.tile_pool(name="wps", bufs=1, space="PSUM") as wps,
        ):
            # PE p-state warmup: matmuls on a zeroed scratch tile, no DMA
            # dependency, so the HAM ramp starts as early as possible.
            # GPSIMD does the tiny memsets (it is the first engine free
            # after the tile-context preamble barrier).
            scratch = singles.tile([128, 128], fp8)
            nc.gpsimd.memset(scratch[:, :], 0)
            bf16 = mybir.dt.bfloat16
            ones = singles.tile([128, 1], bf16)
            nc.gpsimd.memset(ones[:, :], 1.0)
            wm_psum = wps.tile([128, 128], f32)
            for _ in range(WARMUP_MM):
                nc.tensor.matmul(
                    wm_psum[:, :], lhsT=scratch[:, :], rhs=scratch[:, :],
                    start=True, stop=True, skip_group_check=True,
                )

            d_sb = singles.tile([H, H], f32)
            r_psum = rps.tile([128, H], f32)

            # All group loads go on the ONE sync-queue ring: SDMA engines
            # drain a ring FIFO, so groups complete in consumption order.
            # Group 0 also carries the cs column (j=0).
            def load_group(p0, npairs, lead):
                if lead:
                    qbf = qbf_pool.tile([128, 1 + npairs, H], fp8)
                    nc.sync.dma_start(
                        out=qbf[:, :, :], in_=pq[:, 0 : 1 + npairs, :]
                    )
                    return qbf
                qbf = qbf_pool.tile([128, npairs, H], fp8)
                nc.sync.dma_start(
                    out=qbf[:, :, :], in_=pq[:, 1 + p0 : 1 + p0 + npairs, :]
                )
                return qbf

            _state = {"first": True, "cs2": None}

            def emit_y_slab(qbf, ybf, s, off):
                yy = yps.tile([128, SLAB_PAIRS * H], f32)
                sl = slice(off + s * SLAB_PAIRS, off + (s + 1) * SLAB_PAIRS)
                nc.tensor.matmul(
                    yy[:, :], lhsT=_state["cs2"], rhs=qbf[:, sl, :],
                    start=True, stop=True, skip_group_check=True,
                )
                # PSUM -> SBUF fp8 cast: whole slab on ONE engine,
                # alternating DVE/ACT per slab (the ~240ns fixed
                # instruction overhead dominates half-slab casts, which
                # run both engines at ~100% occupancy and gate R).
                eng = nc.vector.tensor_copy if s % 2 == 0 else nc.scalar.copy
                eng(ybf[:, s * SLAB_PAIRS : (s + 1) * SLAB_PAIRS, :], yy[:, :])

            def emit_r_block(qbf, ybf, k, off, last):
                from concourse import mybir as mb

                first = _state["first"]
                _state["first"] = False
                nc.tensor.matmul(
                    r_psum[:, :],
                    lhsT=qbf[:, off + k : off + k + 2, :],
                    rhs=ybf[:, k : k + 2, :],
                    start=first, stop=last, skip_group_check=True,
                    perf_mode=mb.MatmulPerfMode.DoubleRow,
                )

            # Software pipeline: R-matmuls run one group behind the
            # Y-matmuls so the PE never waits on the PSUM->SBUF casts.
            assert sum(GROUPS) == NPAIRS
            prev = None
            p0 = 0
            for gi, npairs in enumerate(GROUPS):
                qbf = load_group(p0, npairs, lead=(gi == 0))
                off = 1 if gi == 0 else 0
                if gi == 0:
                    _state["cs2"] = qbf[:, 0, :]
                p0 += npairs
                ybf = ybf_pool.tile([128, npairs, H], fp8)
                for s in range(npairs // SLAB_PAIRS):
                    emit_y_slab(qbf, ybf, s, off)
                if prev is not None:
                    pq_, py_, po_, pn_ = prev
                    for k in range(0, pn_, 2):
                        emit_r_block(pq_, py_, k, po_, last=False)
                if gi == D_LOAD_AFTER:
                    nc.sync.dma_start(out=d_sb[:, :], in_=dmat[:, :])
                prev = (qbf, ybf, off, npairs)
            pq_, py_, po_, pn_ = prev
            for k in range(0, pn_, 2):
                emit_r_block(pq_, py_, k, po_, last=(k == pn_ - 2))

            # On-device <d_error, R>: per-partition dot on DVE, then a
            # 1-column f32 matmul against ones folds the partition axis.
            prod = singles.tile([128, H], f32)
            acc16 = singles.tile([128, 1], bf16)
            nc.vector.scalar_tensor_tensor(
                out=prod[:, :], in0=r_psum[:, :], scalar=1.0, in1=d_sb[:, :],
                op0=mybir.AluOpType.mult, op1=mybir.AluOpType.mult,
                accum_out=acc16[:, :],
            )
            nc.tensor.matmul(
                wm_psum[0:1, 0:1], lhsT=ones[:, :], rhs=acc16[:, :],
                start=True, stop=True, skip_group_check=True,
            )
            res_sb = singles.tile([1, 1], f32)
            nc.vector.tensor_copy(res_sb[:, :], wm_psum[0:1, 0:1])
            nc.sync.dma_start(out=res_out[:, :], in_=res_sb[:, :])

    nc.compile()
    return nc


def get_nc():
    key = ("nc", "fp8v2")
    if key not in _CACHE:
        _CACHE[key] = _build()
    return _CACHE[key]


def make_count_matrix(circuit_edge_pairs):
    pairs = np.asarray(circuit_edge_pairs).astype(np.int64)
    C = np.zeros((L, L), np.float64)
    np.add.at(C, (pairs[:, 0], pairs[:, 1]), 1.0)
    Cs = (C + C.T) * 0.5
    cs8 = Cs.astype(ml_dtypes.float8_e4m3)
    bd = np.zeros((128, 128), ml_dtypes.float8_e4m3)
    bd[:L, :L] = cs8
    bd[L:, L:] = cs8
    return bd


def pack_shard(Q, csb):
    """(256, 64, 128) f32 -> (128, 129, 128) fp8 with column 0 = csb and
    T[p, 1+j, h] = 64*Q[2j + p//64, p%64, h]."""
    arr = (Q.reshape(NPAIRS, 2, L, H) * QSCALE).astype(ml_dtypes.float8_e4m3)
    out = np.empty((128, 1 + NPAIRS, H), ml_dtypes.float8_e4m3)
    out[:, 0, :] = csb
    out[:, 1:, :] = arr.transpose(1, 2, 0, 3).reshape(128, NPAIRS, H)
    return out


def make_in_maps(P, d_error, circuit_edge_pairs):
    P = np.asarray(P)
    csb = make_count_matrix(circuit_edge_pairs)
    dmat = np.ascontiguousarray(np.asarray(d_error), dtype=np.float32)
    in_maps = []
    for c in range(N_CORES):
        shard = np.ascontiguousarray(
            P[c * BPC : (c + 1) * BPC, :L, :], dtype=np.float32
        )
        in_maps.append({"pq": pack_shard(shard, csb), "dmat": dmat})
    return in_maps


def reduce_results(per_core_res, d_error=None):
    total = 0.0
    for r in per_core_res:
        total += float(np.asarray(r).reshape(()))
    out = total / (B * E * QSCALE * QSCALE)
    return np.array(out, dtype=np.float32)


def run_spmd(P, d_error, circuit_edge_pairs, **kwargs):
    """Run on the 8 NeuronCores; returns (per-core scalars, BassKernelResults)."""
    from concourse.bass_utils import run_bass_kernel_spmd

    nc = get_nc()
    in_maps = make_in_maps(P, d_error, circuit_edge_pairs)
    res = run_bass_kernel_spmd(nc, in_maps, core_ids=list(range(N_CORES)), **kwargs)
    per_core = [res.results[c]["res_out"] for c in range(N_CORES)]
    return per_core, res


def kernel(P, d_error, circuit_edge_pairs, num_logical):
    assert int(num_logical) == L
    per_core, _ = run_spmd(P, d_error, circuit_edge_pairs)
    return reduce_results(per_core)


# revision 10
# speedup vs baseline: 1.0602x; 1.0602x over previous
"""Trainium2 Bass kernel for nn_ErrorAwareEdgeLoss.

reference:  cost[b,e] = sum_{p,q} P[b,i_e,p] * d_error[p,q] * P[b,j_e,q]
            result    = mean_{b,e} cost[b,e]

The edge pairs only enter through the count matrix
    C[l1,l2] = #edges e with (i_e,j_e) == (l1,l2),
and since d_error is symmetric the result collapses to
    result = <d_error, sum_b Q_b^T Cs Q_b> / (B*E),
with Q_b = P[b,:64,:] and Cs = (C + C^T)/2 (exact in fp8: half-integers).

Device work per core (256 batches, data-parallel over batch), all in fp8
(e4m3, host-packed as 64*Q — the softmax values are tiny so 64*Q stays
well inside [2^-6, 240] and the final scale divides by 64^2; empirical
rel err vs f64 reference ~2e-3, tolerance 2e-2):
  - warmup matmuls on a zeroed scratch tile start the PE before any DMA
    lands (the HAM clock gate needs ~3.4us of sustained PE activity to
    lift the PE from 1.2 to 2.4GHz); scratch is memset on GPSIMD, which
    is idle after the preamble, so the PE starts ~0.7us earlier than a
    DVE memset allows
  - cs (blockdiag(Cs,Cs), so one matmul computes both batch-halves' Y)
    is packed as column 0 of the pq dram tensor: the lead-in group load
    brings {cs | first pairs} in ONE descriptor batch on the sync ring,
    saving a ~780ns DIRECT2D on the lead-in critical path
  - group loads all go on the ONE sync HWDGE ring so they complete
    strictly in consumption order; [128, pairs, 128] fp8 tiles, two
    batches per 128 partitions
  - Y = Cs @ Q via ONE blockdiag(Cs,Cs) matmul per 512-wide slab
  - cast Y (PSUM f32) to fp8 in SBUF as whole 512-col slabs alternating
    between DVE and ACT (fixed per-instruction overhead dominates
    smaller casts)
  - R += Q^T Y with K=256 fp8 DoubleRow matmuls (two batch-pairs per
    instruction) accumulated in PSUM f32 — R-matmuls run one group
    behind the Y-matmuls so the PE never waits on the PSUM->SBUF casts
  - final reduce ON DEVICE: s[p] = sum_q R[p,q]*d_error[p,q] via one
    DVE tensor_tensor_reduce (PSUM x SBUF), then a 1-column f32 matmul
    against a ones vector folds partitions, and a single 4-byte result
    is written out — replacing the 64KB R writeback (+copies +big
    descriptor batch) that used to pace the tail.
Host: result = sum_c scalar_c / (B*E*64^2).
"""

import sys

_TRN_REPO = "/opt/trn_rl_repo"
if _TRN_REPO not in sys.path:
    sys.path.insert(0, _TRN_REPO)

import numpy as np
import ml_dtypes

B, L, H = 2048, 64, 128     # batch, logical qubits, physical dim
E = 512                     # number of circuit edges
N_CORES = 8
BPC = B // N_CORES          # 256 batches per core
NPAIRS = BPC // 2           # 128 batch-pairs per core
GROUPS = [8, 8] + [16] * 6 + [8, 8]  # pairs per load: small lead-in groups
                            # start the PE sooner, small tail groups cut
                            # the trailing R run; sum must be NPAIRS
SLAB_PAIRS = 4              # pairs per Y-matmul slab (512 moving columns)
QSCALE = 64.0               # host-side scale before fp8 cast
WARMUP_MM = 22              # dummy matmuls that keep the PE busy (ramping
                            # the HAM clock gate) until the first group
                            # load lands
D_LOAD_AFTER = 7            # ring position of the d_error load (late
                            # enough not to delay the lead-in groups,
                            # early enough to land before the final
                            # reduce)

_CACHE = {}


def _build():
    import concourse.tile as tile
    from concourse import bacc, mybir

    f32 = mybir.dt.float32
    fp8 = mybir.dt.float8e4

    nc = bacc.Bacc(None)
    # host-packed shard: pq[p, 0, h] = blockdiag(Cs,Cs)[p, h] and
    # pq[p, 1+j, h] = 64*Q[2j + p//64, p%64, h] over the core's 128
    # batch-pairs j — each group load is a plain 2D DMA with a
    # contiguous run per partition.
    pq = nc.dram_tensor("pq", [128, 1 + NPAIRS, H], fp8, kind="ExternalInput")
    dmat = nc.dram_tensor("dmat", [H, H], f32, kind="ExternalInput")
    res_out = nc.dram_tensor("res_out", [1, 1], f32, kind="ExternalOutput")

    with tile.TileContext(nc) as tc:
        with (
            tc.tile_pool(name="singles", bufs=1) as singles,
            tc.tile_pool(name="qbfp", bufs=10) as qbf_pool,
            tc.tile_pool(name="ybfp", bufs=3) as ybf_pool,
            tc.tile_pool(name="yps", bufs=6, space="PSUM") as yps,
            tc.tile_pool(name="rps", bufs=1, space="PSUM") as rps,
            tc.tile_pool(name="wps", bufs=1, space="PSUM") as wps,
        ):
            # PE p-state warmup: matmuls on a zeroed scratch tile, no DMA
            # dependency, so the HAM ramp starts as early as possible.
            # GPSIMD does the tiny memsets (it is the first engine free
            # after the tile-context preamble barrier).
            scratch = singles.tile([128, 128], fp8)
            nc.gpsimd.memset(scratch[:, :], 0)
            bf16 = mybir.dt.bfloat16
            ones = singles.tile([128, 1], bf16)
            nc.gpsimd.memset(ones[:, :], 1.0)
            wm_psum = wps.tile([128, 128], f32)
            for _ in range(WARMUP_MM):
                nc.tensor.matmul(
                    wm_psum[:, :], lhsT=scratch[:, :], rhs=scratch[:, :],
                    start=True, stop=True, skip_group_check=True,
                )

            d_sb = singles.tile([H, H], f32)
            r_psum = rps.tile([128, H], f32)

            # All group loads go on the ONE sync-queue ring: SDMA engines
            # drain a ring FIFO, so groups complete in consumption order.
            # Group 0 also carries the cs column (j=0).
            def load_group(p0, npairs, lead):
                if lead:
                    qbf = qbf_pool.tile([128, 1 + npairs, H], fp8)
                    nc.sync.dma_start(
                        out=qbf[:, :, :], in_=pq[:, 0 : 1 + npairs, :]
                    )
                    return qbf
                qbf = qbf_pool.tile([128, npairs, H], fp8)
                nc.sync.dma_start(
                    out=qbf[:, :, :], in_=pq[:, 1 + p0 : 1 + p0 + npairs, :]
                )
                return qbf

            _state = {"first": True, "cs2": None}

            def emit_y_slab(qbf, ybf, s, off):
                yy = yps.tile([128, SLAB_PAIRS * H], f32)
                sl = slice(off + s * SLAB_PAIRS, off + (s + 1) * SLAB_PAIRS)
                nc.tensor.matmul(
                    yy[:, :], lhsT=_state["cs2"], rhs=qbf[:, sl, :],
                    start=True, stop=True, skip_group_check=True,
                )
                # PSUM -> SBUF fp8 cast: whole slab on ONE engine,
                # alternating DVE/ACT per slab (the ~240ns fixed
                # instruction overhead dominates half-slab casts, which
                # run both engines at ~100% occupancy and gate R).
                eng = nc.vector.tensor_copy if s % 2 == 0 else nc.scalar.copy
                eng(ybf[:, s * SLAB_PAIRS : (s + 1) * SLAB_PAIRS, :], yy[:, :])

            def emit_r_block(qbf, ybf, k, off, last):
                from concourse import mybir as mb

                first = _state["first"]
                _state["first"] = False
                nc.tensor.matmul(
                    r_psum[:, :],
                    lhsT=qbf[:, off + k : off + k + 2, :],
                    rhs=ybf[:, k : k + 2, :],
                    start=first, stop=last, skip_group_check=True,
                    perf_mode=mb.MatmulPerfMode.DoubleRow,
                )

            # Software pipeline: R-matmuls run one group behind the
            # Y-matmuls so the PE never waits on the PSUM->SBUF casts.
            assert sum(GROUPS) == NPAIRS
            prev = None
            p0 = 0
            for gi, npairs in enumerate(GROUPS):
                qbf = load_group(p0, npairs, lead=(gi == 0))
                off = 1 if gi == 0 else 0
                if gi == 0:
                    _state["cs2"] = qbf[:, 0, :]
                p0 += npairs
                ybf = ybf_pool.tile([128, npairs, H], fp8)
                for s in range(npairs // SLAB_PAIRS):
                    emit_y_slab(qbf, ybf, s, off)
                if prev is not None:
                    pq_, py_, po_, pn_ = prev
                    for k in range(0, pn_, 2):
                        emit_r_block(pq_, py_, k, po_, last=False)
                if gi == D_LOAD_AFTER:
                    # software DGE on the idle GPSIMD engine: keeps the
                    # 64KB d_error load entirely off the sync ring that
                    # feeds the PE
                    nc.gpsimd.dma_start(out=d_sb[:, :], in_=dmat[:, :])
                prev = (qbf, ybf, off, npairs)
            pq_, py_, po_, pn_ = prev
            for k in range(0, pn_, 2):
                emit_r_block(pq_, py_, k, po_, last=(k == pn_ - 2))

            # On-device <d_error, R>: per-partition dot on DVE, then a
            # 1-column f32 matmul against ones folds the partition axis.
            prod = singles.tile([128, H], f32)
            acc16 = singles.tile([128, 1], bf16)
            nc.vector.scalar_tensor_tensor(
                out=prod[:, :], in0=r_psum[:, :], scalar=1.0, in1=d_sb[:, :],
                op0=mybir.AluOpType.mult, op1=mybir.AluOpType.mult,
                accum_out=acc16[:, :],
            )
            nc.tensor.matmul(
                wm_psum[0:1, 0:1], lhsT=ones[:, :], rhs=acc16[:, :],
                start=True, stop=True, skip_group_check=True,
            )
            res_sb = singles.tile([1, 1], f32)
            nc.vector.tensor_copy(res_sb[:, :], wm_psum[0:1, 0:1])
            nc.sync.dma_start(out=res_out[:, :], in_=res_sb[:, :])

    nc.compile()
    return nc


def get_nc():
    key = ("nc", "fp8v2")
    if key not in _CACHE:
        _CACHE[key] = _build()
    return _CACHE[key]


def make_count_matrix(circuit_edge_pairs):
    pairs = np.asarray(circuit_edge_pairs).astype(np.int64)
    C = np.zeros((L, L), np.float64)
    np.add.at(C, (pairs[:, 0], pairs[:, 1]), 1.0)
    Cs = (C + C.T) * 0.5
    cs8 = Cs.astype(ml_dtypes.float8_e4m3)
    bd = np.zeros((128, 128), ml_dtypes.float8_e4m3)
    bd[:L, :L] = cs8
    bd[L:, L:] = cs8
    return bd


def pack_shard(Q, csb):
    """(256, 64, 128) f32 -> (128, 129, 128) fp8 with column 0 = csb and
    T[p, 1+j, h] = 64*Q[2j + p//64, p%64, h]."""
    arr = (Q.reshape(NPAIRS, 2, L, H) * QSCALE).astype(ml_dtypes.float8_e4m3)
    out = np.empty((128, 1 + NPAIRS, H), ml_dtypes.float8_e4m3)
    out[:, 0, :] = csb
    out[:, 1:, :] = arr.transpose(1, 2, 0, 3).reshape(128, NPAIRS, H)
    return out


def make_in_maps(P, d_error, circuit_edge_pairs):
    P = np.asarray(P)
    csb = make_count_matrix(circuit_edge_pairs)
    dmat = np.ascontiguousarray(np.asarray(d_error), dtype=np.float32)
    in_maps = []
    for c in range(N_CORES):
        shard = np.ascontiguousarray(
            P[c * BPC : (c + 1) * BPC, :L, :], dtype=np.float32
        )
        in_maps.append({"pq": pack_shard(shard, csb), "dmat": dmat})
    return in_maps


def reduce_results(per_core_res, d_error=None):
    total = 0.0
    for r in per_core_res:
        total += float(np.asarray(r).reshape(()))
    out = total / (B * E * QSCALE * QSCALE)
    return np.array(out, dtype=np.float32)


def run_spmd(P, d_error, circuit_edge_pairs, **kwargs):
    """Run on the 8 NeuronCores; returns (per-core scalars, BassKernelResults)."""
    from concourse.bass_utils import run_bass_kernel_spmd

    nc = get_nc()
    in_maps = make_in_maps(P, d_error, circuit_edge_pairs)
    res = run_bass_kernel_spmd(nc, in_maps, core_ids=list(range(N_CORES)), **kwargs)
    per_core = [res.results[c]["res_out"] for c in range(N_CORES)]
    return per_core, res


def kernel(P, d_error, circuit_edge_pairs, num_logical):
    assert int(num_logical) == L
    per_core, _ = run_spmd(P, d_error, circuit_edge_pairs)
    return reduce_results(per_core)
